# revision 28
# baseline (speedup 1.0000x reference)
"""GA3 Conv2d kernel for 8 Trainium2 NeuronCores.

Math: the reference computes, per batch image,
    out[b, co, m] = sum_{j,k} S[m,j,k] * (conv2d(a_k, W[j]) + bias[j])[co]
with a_k = x[:, k::8] (blade-interleaved channels).  Because the sign
combination is linear, it folds into the conv weights:
    V[co*8+m, ci*8+k, kh, kw] = sum_j S[m,j,k] * W[j, co, ci, kh, kw]
    bias_eff[co*8+m]          = sum_{j,k} S[m,j,k] * b[j, co]
so the whole module is ONE dense 3x3 conv with Cin=Cout=128 on
[B, 128, 128, 128].  We shard data-parallel over B across the 8 cores
(1 image per core) and implement the conv as 9 shifted fp16 matmuls per
4-row output block (tap weights stationary [ic=128 x oc=128], pixels
moving, fp32 PSUM accumulation; fp16 operands keep LDWEIGHTS fully
hidden behind the 512-column matmul stream and halve input DMA bytes;
measured rel err ~3e-4).

Layout: the host pre-pads each image into a flat per-partition buffer
    [pad pair][row: 128 data + pad pair] x 130 padded rows
(pitch 130, zeros at the halo) so every device-side load is a fully
contiguous DMA and tap shifts are pure address offsets.  The zero halo
rows are memset on-device (GpSimd) instead of DMA'd.  Loads ride the SP
HWDGE ring in FIFO order — first data chunk, then the weight table split
so the first taps' columns land early; the early-phase DMA rate is the
8-core-contended bottleneck that gates the first real matmul (~10us).
Warm-up matmuls on a GpSimd-memset scratch tile run during the head DMAs
so the PE HAM clock gate is at 2.4 GHz when the real matmuls start.
Output is stored as fp16 (halves store traffic; host upconverts), with
per-strip stores on the ACT ring and a fine-grained final strip (4-row
then 2-row groups, stores alternating ACT/SP rings) so the post-last-
matmul chain is one small drain + one small store.  Measured ~78.5us vs
the 81.0us baseline; the stream itself (252 N=512 + 45 smaller matmuls
at ~218ns per N=512 issue) is the 62.4us PE floor for fp16 direct conv
(fp8 DoubleRow would halve it but measures 3.8e-2 rel err vs the 2e-2
gate; int8/uint8 matmul paths are not plumbed in bass/walrus).
"""

import numpy as np

_TERMS = [
    [(0, 0, 1), (1, 1, 1), (2, 2, 1), (3, 3, 1), (4, 4, -1), (5, 5, -1), (6, 6, -1), (7, 7, -1)],
    [(1, 0, 1), (0, 1, 1), (2, 4, 1), (4, 2, -1), (3, 6, 1), (6, 3, -1), (5, 7, -1), (7, 5, -1)],
    [(2, 0, 1), (0, 2, 1), (1, 4, -1), (4, 1, 1), (3, 5, 1), (5, 3, -1), (6, 7, 1), (7, 6, 1)],
    [(3, 0, 1), (0, 3, 1), (1, 6, -1), (6, 1, 1), (2, 5, -1), (5, 2, 1), (4, 7, -1), (7, 4, -1)],
    [(4, 0, 1), (0, 4, 1), (2, 1, 1), (1, 2, -1), (3, 7, 1), (7, 3, 1), (6, 5, 1), (5, 6, -1)],
    [(5, 0, 1), (0, 5, 1), (3, 2, 1), (2, 3, -1), (1, 7, 1), (7, 1, 1), (4, 6, 1), (6, 4, -1)],
    [(6, 0, 1), (0, 6, 1), (3, 1, 1), (1, 3, -1), (2, 7, -1), (7, 2, -1), (5, 4, 1), (4, 5, -1)],
    [(7, 0, 1), (0, 7, 1), (5, 1, 1), (1, 5, 1), (6, 2, -1), (2, 6, -1), (4, 3, 1), (3, 4, 1)],
]
_S = np.zeros((8, 8, 8), dtype=np.float32)
for _m, _terms in enumerate(_TERMS):
    for _j, _k, _s in _terms:
        _S[_m, _j, _k] = _s

B, CIN, COUT, H, W = 8, 16, 16, 128, 128
C = 8 * CIN  # 128 interleaved channels
N_CORES = 8
STRIP = 16          # output rows per strip (one store DMA)
N_STRIPS = H // STRIP
GROUP = 4           # output rows per PSUM accumulation group (4*128 = 512 free)
PW = W + 2          # padded row pitch in the flat layout
NPR = H + 2         # padded rows (-1 .. 128)
FLAT = 2 + NPR * PW  # flat elems/partition: leading pad pair + 130 rows
WCOLS = 9 * C + 1   # packed weight taps + bias column
XOFF = WCOLS        # x flat data starts after the weight columns
N_WARMUP = 5        # HAM warm-up matmuls during the head DMAs

_CACHED_NC = None


def _build_nc():
    import concourse.bass as bass
    import concourse.mybir as mybir
    import concourse.tile as tile
    from concourse import bacc

    f32 = mybir.dt.float32
    f16 = mybir.dt.float16

    nc = bacc.Bacc("TRN2", target_bir_lowering=False, debug=False,
                   enable_asserts=False)

    # Weights and image share one dram tensor: per partition the layout is
    # [9 tap weight cols + bias | flat padded image].  DMAs here are packet-
    # count bound (one packet per partition, ~10ns/packet, size-independent up
    # to 4KB), so packing lets ONE 128-packet DMA deliver the weights plus
    # image rows 0-5 — everything the first PSUM group needs.
    xb = nc.dram_tensor("xb", [C, XOFF + FLAT], f16, kind="ExternalInput").ap()
    out = nc.dram_tensor("out", [C, H, W], f16, kind="ExternalOutput").ap()

    with tile.TileContext(nc) as tc:
        with (
            tc.tile_pool(name="wpool", bufs=1) as wpool,
            tc.tile_pool(name="xpool", bufs=1) as xpool,
            tc.tile_pool(name="pspool", bufs=6, space="PSUM") as pspool,
            tc.tile_pool(name="pstail", bufs=2, space="PSUM") as pstail,
            tc.tile_pool(name="opool", bufs=3) as opool,
        ):
            wx = xpool.tile([C, XOFF + FLAT], f16)
            wtile = wx  # weight cols live at [0:WCOLS); x flat at [XOFF:)

            # HAM warm-up: junk matmuls on a memset scratch tile lift the PE
            # clock gate to 2.4 GHz before the data-gated real matmuls begin.
            # The memset runs on the otherwise-idle GpSimd engine so neither
            # the DVE queue (bias copy waits on the weight DMA) nor the load
            # path delays the warm-up stream.
            wmsrc = wpool.tile([C, 512], f16)
            nc.gpsimd.memset(wmsrc[:, :], 0.0)
            wmps = pspool.tile([C, GROUP * W], f32, tag="ps")
            for _ in range(N_WARMUP):
                nc.tensor.matmul(wmps[:, :], lhsT=wmsrc[:, 0:C],
                                 rhs=wmsrc[:, 0:512], start=True, stop=True,
                                 skip_group_check=True)

            # Loads ride the SP ring in FIFO order.  The first DMA carries the
            # whole weight table + padded rows 0-5 (3870B/partition — still a
            # single packet per partition), unblocking every LDWEIGHTS and the
            # entire first PSUM group ~1.3us after ring-arm.  Remaining image
            # chunks follow at padded-row boundaries; later ones are emitted
            # interleaved with the strips so head DMAs don't collide on the 8
            # shared DMA-completion semaphore lanes.
            bounds = [6, 13, 25, 41, 57, 73, 89, 105, 121, NPR]

            def emit_chunk(c):
                lo = XOFF + 2 + PW * bounds[c]
                hi = XOFF + FLAT if bounds[c + 1] == NPR else \
                    XOFF + 2 + PW * bounds[c + 1]
                nc.sync.dma_start(out=wx[:, lo:hi], in_=xb[:, lo:hi])

            nc.sync.dma_start(out=wx[:, 0:XOFF + 2 + 6 * PW],
                              in_=xb[:, 0:XOFF + 2 + 6 * PW])
            for c in range(3):
                emit_chunk(c)
            # DVE tensor_scalar needs an fp32 scalar operand — up-convert the
            # packed fp16 bias column once
            btile = wpool.tile([C, 1], f32)
            nc.vector.tensor_copy(out=btile[:, :], in_=wtile[:, 9 * C:WCOLS])

            # ---- conv: 32 PSUM groups x 9 shifted matmuls
            def emit_group(row0, nrows, ps, ostart, obuf):
                # output rows row0..row0+nrows read padded rows row0+dh..
                # at column shift dw-1; padded row pr starts at flat 2+130*pr
                ta = 0
                for dh in range(3):
                    for dw in range(3):
                        base = XOFF + 1 + PW * (row0 + dh) + dw
                        rhs = bass.AP(wx.tensor, wx.offset + base,
                                      [wx.ap[0], [PW, nrows], [1, W]])
                        nc.tensor.matmul(
                            ps[:, :],
                            lhsT=wtile[:, ta * C:(ta + 1) * C],
                            rhs=rhs,
                            start=(ta == 0),
                            stop=(ta == 8),
                        )
                        ta += 1
                nc.vector.tensor_scalar_add(
                    out=obuf[:, ostart:ostart + nrows * W],
                    in0=ps[:, :],
                    scalar1=btile[:, 0:1],
                )

            for s in range(N_STRIPS - 1):
                if 1 <= s <= 6:
                    emit_chunk(s + 2)   # stays ~2 strips ahead of consumption
                obuf = opool.tile([C, STRIP * W], f16)
                for g in range(STRIP // GROUP):
                    ps = pspool.tile([C, GROUP * W], f32)
                    emit_group(16 * s + 4 * g, GROUP, ps, g * GROUP * W, obuf)
                nc.scalar.dma_start(
                    out=out[:, s * STRIP:(s + 1) * STRIP, :],
                    in_=obuf[:, :])

            # Final strip: 4-row groups for rows 112-123, then 2-row PSUM
            # groups with per-group stores alternating between the ACT and SP
            # rings, so the post-last-matmul chain is a 2-row drain + a 2-row
            # fp16 store instead of a 4-row drain + a 16-row fp32 stream.
            s = N_STRIPS - 1
            obuf = opool.tile([C, STRIP * W], f16)
            for g in range(3):
                ps = pspool.tile([C, GROUP * W], f32)
                emit_group(16 * s + 4 * g, GROUP, ps, g * GROUP * W, obuf)
                eng = nc.scalar if g % 2 == 0 else nc.sync
                eng.dma_start(
                    out=out[:, s * STRIP + 4 * g:s * STRIP + 4 * (g + 1), :],
                    in_=obuf[:, g * GROUP * W:(g + 1) * GROUP * W])
            for h in range(2):
                ps = pstail.tile([C, 2 * W], f32)
                row0 = 16 * s + 12 + 2 * h
                ostart = (12 + 2 * h) * W
                emit_group(row0, 2, ps, ostart, obuf)
                eng = nc.sync if h % 2 == 0 else nc.scalar
                eng.dma_start(
                    out=out[:, row0:row0 + 2, :],
                    in_=obuf[:, ostart:ostart + 2 * W])

    nc.compile()
    return nc


def _get_nc():
    global _CACHED_NC
    if _CACHED_NC is None:
        _CACHED_NC = _build_nc()
    return _CACHED_NC


def _prep_weights(Wfull: np.ndarray, b: np.ndarray):
    # wf[ic, tap*128 + oc] with ic = ci*8+k, oc = co*8+m, tap = kh*3+kw;
    # final column (index 9*128) holds bias_eff[oc] indexed by partition.
    V = np.einsum("mjk,jcihw->ikhwcm", _S.astype(np.float64),
                  Wfull.astype(np.float64))          # [ci,k,kh,kw,co,m]
    V = V.reshape(C, 9 * C)
    bias = np.einsum("mjk,jc->cm", _S.astype(np.float64),
                     b.astype(np.float64)).reshape(C, 1)
    wf = np.concatenate([V, bias], axis=1)
    return np.ascontiguousarray(wf, dtype=np.float16)


def _pad_images(x: np.ndarray) -> np.ndarray:
    # [B, C, H, W] -> flat padded [B, C, FLAT] (see module docstring)
    xpad = np.zeros((x.shape[0], C, FLAT), dtype=np.float16)
    arr = xpad[:, :, 2:].reshape(x.shape[0], C, NPR, PW)
    arr[:, :, 1:H + 1, 0:W] = x
    return xpad


def _make_in_maps(x: np.ndarray, W: np.ndarray, b: np.ndarray):
    # pack [weights+bias | flat padded image] per partition (see _build_nc)
    xpad = _pad_images(np.ascontiguousarray(x, dtype=np.float32))
    wf = _prep_weights(np.asarray(W), np.asarray(b))
    packed = np.concatenate(
        [np.broadcast_to(wf, (xpad.shape[0], C, WCOLS)), xpad], axis=2)
    packed = np.ascontiguousarray(packed)
    return [{"xb": packed[c]} for c in range(N_CORES)]


def kernel(x: np.ndarray, W: np.ndarray, b: np.ndarray) -> np.ndarray:
    from concourse.bass_utils import run_bass_kernel_spmd

    nc = _get_nc()
    in_maps = _make_in_maps(x, W, b)
    res = run_bass_kernel_spmd(nc, in_maps, core_ids=list(range(N_CORES)))
    return np.stack([res.results[c]["out"] for c in range(N_CORES)],
                    axis=0).astype(np.float32)



# revision 34
# speedup vs baseline: 1.0163x; 1.0163x over previous
"""GA3 Conv2d kernel for 8 Trainium2 NeuronCores.

Math: the reference computes, per batch image,
    out[b, co, m] = sum_{j,k} S[m,j,k] * (conv2d(a_k, W[j]) + bias[j])[co]
with a_k = x[:, k::8] (blade-interleaved channels).  Because the sign
combination is linear, it folds into the conv weights:
    V[co*8+m, ci*8+k, kh, kw] = sum_j S[m,j,k] * W[j, co, ci, kh, kw]
    bias_eff[co*8+m]          = sum_{j,k} S[m,j,k] * b[j, co]
so the whole module is ONE dense 3x3 conv with Cin=Cout=128 on
[B, 128, 128, 128].  We shard data-parallel over B across the 8 cores
(1 image per core) and implement the conv as 9 shifted fp16 matmuls per
4-row output block (tap weights stationary [ic=128 x oc=128], pixels
moving, fp32 PSUM accumulation; fp16 operands keep LDWEIGHTS fully
hidden behind the 512-column matmul stream and halve input DMA bytes;
measured rel err ~3e-4).

Layout: the host pre-pads each image into a flat per-partition buffer
    [pad pair][row: 128 data + pad pair] x 130 padded rows
(pitch 130, zeros at the halo) so every device-side load is a fully
contiguous DMA and tap shifts are pure address offsets.  The zero halo
rows are memset on-device (GpSimd) instead of DMA'd.  Loads ride the SP
HWDGE ring in FIFO order — first data chunk, then the weight table split
so the first taps' columns land early; the early-phase DMA rate is the
8-core-contended bottleneck that gates the first real matmul (~10us).
Warm-up matmuls on a GpSimd-memset scratch tile run during the head DMAs
so the PE HAM clock gate is at 2.4 GHz when the real matmuls start.
Output is stored as fp16 (halves store traffic; host upconverts), with
per-strip stores on the ACT ring and a fine-grained final strip (4-row
then 2-row groups, stores alternating ACT/SP rings) so the post-last-
matmul chain is one small drain + one small store.  Measured ~78.5us vs
the 81.0us baseline; the stream itself (252 N=512 + 45 smaller matmuls
at ~218ns per N=512 issue) is the 62.4us PE floor for fp16 direct conv
(fp8 DoubleRow would halve it but measures 3.8e-2 rel err vs the 2e-2
gate; int8/uint8 matmul paths are not plumbed in bass/walrus).
"""

import numpy as np

_TERMS = [
    [(0, 0, 1), (1, 1, 1), (2, 2, 1), (3, 3, 1), (4, 4, -1), (5, 5, -1), (6, 6, -1), (7, 7, -1)],
    [(1, 0, 1), (0, 1, 1), (2, 4, 1), (4, 2, -1), (3, 6, 1), (6, 3, -1), (5, 7, -1), (7, 5, -1)],
    [(2, 0, 1), (0, 2, 1), (1, 4, -1), (4, 1, 1), (3, 5, 1), (5, 3, -1), (6, 7, 1), (7, 6, 1)],
    [(3, 0, 1), (0, 3, 1), (1, 6, -1), (6, 1, 1), (2, 5, -1), (5, 2, 1), (4, 7, -1), (7, 4, -1)],
    [(4, 0, 1), (0, 4, 1), (2, 1, 1), (1, 2, -1), (3, 7, 1), (7, 3, 1), (6, 5, 1), (5, 6, -1)],
    [(5, 0, 1), (0, 5, 1), (3, 2, 1), (2, 3, -1), (1, 7, 1), (7, 1, 1), (4, 6, 1), (6, 4, -1)],
    [(6, 0, 1), (0, 6, 1), (3, 1, 1), (1, 3, -1), (2, 7, -1), (7, 2, -1), (5, 4, 1), (4, 5, -1)],
    [(7, 0, 1), (0, 7, 1), (5, 1, 1), (1, 5, 1), (6, 2, -1), (2, 6, -1), (4, 3, 1), (3, 4, 1)],
]
_S = np.zeros((8, 8, 8), dtype=np.float32)
for _m, _terms in enumerate(_TERMS):
    for _j, _k, _s in _terms:
        _S[_m, _j, _k] = _s

B, CIN, COUT, H, W = 8, 16, 16, 128, 128
C = 8 * CIN  # 128 interleaved channels
N_CORES = 8
STRIP = 16          # output rows per strip (one store DMA)
N_STRIPS = H // STRIP
GROUP = 4           # output rows per PSUM accumulation group (4*128 = 512 free)
PW = W + 2          # padded row pitch in the flat layout
NPR = H + 2         # padded rows (-1 .. 128)
FLAT = 2 + NPR * PW  # flat elems/partition: leading pad pair + 130 rows
WCOLS = 9 * C + 1   # packed weight taps + bias column
N_WARMUP = 6        # HAM warm-up matmuls during the head DMAs

_CACHED_NC = None


def _build_nc():
    import concourse.bass as bass
    import concourse.mybir as mybir
    import concourse.tile as tile
    from concourse import bacc

    f32 = mybir.dt.float32
    f16 = mybir.dt.float16

    nc = bacc.Bacc("TRN2", target_bir_lowering=False, debug=False,
                   enable_asserts=False)

    xb = nc.dram_tensor("xb", [C, FLAT], f16, kind="ExternalInput").ap()
    wf = nc.dram_tensor("wf", [C, WCOLS], f16, kind="ExternalInput").ap()
    out = nc.dram_tensor("out", [C, H, W], f16, kind="ExternalOutput").ap()

    with tile.TileContext(nc) as tc:
        with (
            tc.tile_pool(name="wpool", bufs=1) as wpool,
            tc.tile_pool(name="xpool", bufs=1) as xpool,
            tc.tile_pool(name="pspool", bufs=6, space="PSUM") as pspool,
            tc.tile_pool(name="pstail", bufs=2, space="PSUM") as pstail,
            tc.tile_pool(name="opool", bufs=3) as opool,
        ):
            xfull = xpool.tile([C, FLAT], f16)
            wtile = wpool.tile([C, WCOLS], f16)

            # HAM warm-up: junk matmuls on a memset scratch tile lift the PE
            # clock gate to 2.4 GHz before the data-gated real matmuls begin.
            # The memset runs on the otherwise-idle GpSimd engine so neither
            # the DVE queue (bias copy waits on the weight DMA) nor the load
            # path delays the warm-up stream.
            wmsrc = wpool.tile([C, 512], f16)
            nc.gpsimd.memset(wmsrc[:, :], 0.0)
            # The zero halo rows (padded rows 0 and 129 + the lead pad pair)
            # are memset on-device instead of being DMA'd from HBM — this
            # trims the critical first chunk and runs on the idle GpSimd
            # engine long before any matmul reads them.
            nc.gpsimd.memset(xfull[:, 0:132], 0.0)
            nc.gpsimd.memset(xfull[:, 2 + 129 * PW:FLAT], 0.0)
            wmps = pspool.tile([C, GROUP * W], f32, tag="ps")
            for _ in range(N_WARMUP):
                nc.tensor.matmul(wmps[:, :], lhsT=wmsrc[:, 0:C],
                                 rhs=wmsrc[:, 0:512], start=True, stop=True,
                                 skip_group_check=True)

            # input chunks (contiguous flat ranges at padded-row boundaries).
            # All loads ride the SP ring in FIFO order; the head is byte-bound
            # at the contended early DMA rate, so the gate is kept small: the
            # first chunk plus the first taps' weight columns, with the rest
            # of the weight table right behind.  Later chunks are emitted
            # interleaved with the strips so head DMAs don't collide on the 8
            # shared DMA-completion semaphore lanes.
            bounds = [0, 5, 9, 13, 25, 41, 57, 73, 89, 105, 121, NPR]

            def emit_chunk(c):
                # rows 0 and 129 are zero halo — memset above, never DMA'd
                a, b = max(bounds[c], 1), min(bounds[c + 1], NPR - 1)
                lo, hi = 2 + PW * a, 2 + PW * b
                nc.sync.dma_start(out=xfull[:, lo:hi], in_=xb[:, lo:hi])

            emit_chunk(0)
            nc.sync.dma_start(out=wtile[:, 0:3 * C], in_=wf[:, 0:3 * C])
            nc.sync.dma_start(out=wtile[:, 3 * C:WCOLS], in_=wf[:, 3 * C:WCOLS])
            for c in range(1, 5):
                emit_chunk(c)
            # DVE tensor_scalar needs an fp32 scalar operand — up-convert the
            # packed fp16 bias column once
            btile = wpool.tile([C, 1], f32)
            nc.vector.tensor_copy(out=btile[:, :], in_=wtile[:, 9 * C:WCOLS])

            # ---- conv: 32 PSUM groups x 9 shifted matmuls
            def emit_group(row0, nrows, ps, ostart, obuf):
                # output rows row0..row0+nrows read padded rows row0+dh..
                # at column shift dw-1; padded row pr starts at flat 2+130*pr
                ta = 0
                for dh in range(3):
                    for dw in range(3):
                        base = 1 + PW * (row0 + dh) + dw
                        rhs = bass.AP(xfull.tensor, xfull.offset + base,
                                      [xfull.ap[0], [PW, nrows], [1, W]])
                        nc.tensor.matmul(
                            ps[:, :],
                            lhsT=wtile[:, ta * C:(ta + 1) * C],
                            rhs=rhs,
                            start=(ta == 0),
                            stop=(ta == 8),
                        )
                        ta += 1
                nc.vector.tensor_scalar_add(
                    out=obuf[:, ostart:ostart + nrows * W],
                    in0=ps[:, :],
                    scalar1=btile[:, 0:1],
                )

            for s in range(N_STRIPS - 1):
                if 1 <= s <= 6:
                    emit_chunk(s + 4)   # stays ~2 strips ahead of consumption
                obuf = opool.tile([C, STRIP * W], f16)
                for g in range(STRIP // GROUP):
                    ps = pspool.tile([C, GROUP * W], f32)
                    emit_group(16 * s + 4 * g, GROUP, ps, g * GROUP * W, obuf)
                nc.scalar.dma_start(
                    out=out[:, s * STRIP:(s + 1) * STRIP, :],
                    in_=obuf[:, :])

            # Final strip: 4-row groups for rows 112-123, then 2-row PSUM
            # groups with per-group stores alternating between the ACT and SP
            # rings, so the post-last-matmul chain is a 2-row drain + a 2-row
            # fp16 store instead of a 4-row drain + a 16-row fp32 stream.
            s = N_STRIPS - 1
            obuf = opool.tile([C, STRIP * W], f16)
            for g in range(3):
                ps = pspool.tile([C, GROUP * W], f32)
                emit_group(16 * s + 4 * g, GROUP, ps, g * GROUP * W, obuf)
                eng = nc.scalar if g % 2 == 0 else nc.sync
                eng.dma_start(
                    out=out[:, s * STRIP + 4 * g:s * STRIP + 4 * (g + 1), :],
                    in_=obuf[:, g * GROUP * W:(g + 1) * GROUP * W])
            for h in range(2):
                ps = pstail.tile([C, 2 * W], f32)
                row0 = 16 * s + 12 + 2 * h
                ostart = (12 + 2 * h) * W
                emit_group(row0, 2, ps, ostart, obuf)
                eng = nc.sync if h % 2 == 0 else nc.scalar
                eng.dma_start(
                    out=out[:, row0:row0 + 2, :],
                    in_=obuf[:, ostart:ostart + 2 * W])

    nc.compile()
    return nc


def _get_nc():
    global _CACHED_NC
    if _CACHED_NC is None:
        _CACHED_NC = _build_nc()
    return _CACHED_NC


def _prep_weights(Wfull: np.ndarray, b: np.ndarray):
    # wf[ic, tap*128 + oc] with ic = ci*8+k, oc = co*8+m, tap = kh*3+kw;
    # final column (index 9*128) holds bias_eff[oc] indexed by partition.
    V = np.einsum("mjk,jcihw->ikhwcm", _S.astype(np.float64),
                  Wfull.astype(np.float64))          # [ci,k,kh,kw,co,m]
    V = V.reshape(C, 9 * C)
    bias = np.einsum("mjk,jc->cm", _S.astype(np.float64),
                     b.astype(np.float64)).reshape(C, 1)
    wf = np.concatenate([V, bias], axis=1)
    return np.ascontiguousarray(wf, dtype=np.float16)


def _pad_images(x: np.ndarray) -> np.ndarray:
    # [B, C, H, W] -> flat padded [B, C, FLAT] (see module docstring)
    xpad = np.zeros((x.shape[0], C, FLAT), dtype=np.float16)
    arr = xpad[:, :, 2:].reshape(x.shape[0], C, NPR, PW)
    arr[:, :, 1:H + 1, 0:W] = x
    return xpad


def _make_in_maps(x: np.ndarray, W: np.ndarray, b: np.ndarray):
    xpad = _pad_images(np.ascontiguousarray(x, dtype=np.float32))
    wf = _prep_weights(np.asarray(W), np.asarray(b))
    return [{"xb": xpad[c], "wf": wf} for c in range(N_CORES)]


def kernel(x: np.ndarray, W: np.ndarray, b: np.ndarray) -> np.ndarray:
    from concourse.bass_utils import run_bass_kernel_spmd

    nc = _get_nc()
    in_maps = _make_in_maps(x, W, b)
    res = run_bass_kernel_spmd(nc, in_maps, core_ids=list(range(N_CORES)))
    return np.stack([res.results[c]["out"] for c in range(N_CORES)],
                    axis=0).astype(np.float32)

